# revision 21
# baseline (speedup 1.0000x reference)
"""ModalityUntiedAttention on 8 TRN2 NeuronCores (Bass/Tile).

Sharding: data-parallel over batch (cores 0-3 -> batch 0, cores 4-7 -> batch 1),
tensor-parallel over heads within each 4-core group (4 q heads + 2 kv heads per
core). Modality routing via delta weights: out = x@W0 + m*(x@Wd) with the
per-token 0/1 mask applied as a per-partition scalar at PSUM eviction.
Attention is computed with keys on partitions (scores^T), softmax without max
subtraction (|scores| <= sqrt(128) with unit norm weights), denominator via a
ones-column matmul, causal mask via affine_select. Final wo projection partial
sums are ReduceScattered over each 4-core group (chunked, 4 chunks of 512
tokens) and RMSNormed on device.
"""
import sys

sys.path.insert(0, '/opt/trn_rl_repo')

from contextlib import ExitStack

import numpy as np
import ml_dtypes

import concourse.bass as bass
import concourse.tile as tile
from concourse.bass import _add_dep_helper
from concourse import bacc, mybir
from concourse.bass import ts, ds
from concourse.bass_utils import run_bass_kernel_spmd
from concourse.masks import make_identity

F32 = mybir.dt.float32
F32R = mybir.dt.float32r
BF16 = mybir.dt.bfloat16

E = 2
HQ = 16
HK = 8
HD = 128
DIM = 2048
BS = 2
SEQ = 2048
EPS = 1e-6

N_CORES = 8
TP = 4                     # cores per batch group
HQC = HQ // TP             # 4 q heads per core
HKC = HK // TP             # 2 kv heads per core
DQ = HQC * HD              # 512 q cols per core
DKV = HKC * HD             # 256 k (and v) cols per core
NT = SEQ // 128            # 16 token tiles
KT = DIM // 128            # 16 contraction tiles
NG = 4                     # q groups of 512 tokens (chunks for RS)
GROUPS = [[0, 1, 2, 3], [4, 5, 6, 7]]

_BUILD_CACHE = {}

import os
ATT_DT_NAME = os.environ.get("ATT_DT", "bf16")
STAGE = os.environ.get("STAGE", "full")   # p1 | p2 | p3nc | full


def _r(ap):
    """matmul view of f32 tiles as float32r (1 cyc/row at N>=256)."""
    return ap.bitcast(F32R)


def build_nc(has_qkw: bool, has_anw: bool):
    ATT = {"bf16": BF16, "f32r": F32R}[ATT_DT_NAME]
    nc = bacc.Bacc("TRN2", target_bir_lowering=False, debug=False,
                   num_devices=N_CORES)

    xT = nc.dram_tensor("xT", [DIM, SEQ], BF16, kind="ExternalInput")
    w0 = nc.dram_tensor("w0", [DIM, DQ + DKV + DKV], BF16, kind="ExternalInput")
    wd = nc.dram_tensor("wd", [DIM, DQ + DKV + DKV], BF16, kind="ExternalInput")
    wo0 = nc.dram_tensor("wo0", [DQ, DIM], BF16, kind="ExternalInput")
    wod = nc.dram_tensor("wod", [DQ, DIM], BF16, kind="ExternalInput")
    cosf = nc.dram_tensor("cosf", [SEQ, HD], F32, kind="ExternalInput")
    sinf = nc.dram_tensor("sinf", [SEQ, HD], F32, kind="ExternalInput")
    mpc = nc.dram_tensor("mpc", [128, NT], F32, kind="ExternalInput")
    mrow = nc.dram_tensor("mrow", [1, SEQ], BF16, kind="ExternalInput")
    if has_qkw:
        # per-token modality-selected q/k rmsnorm weights, head-tiled
        qkw = nc.dram_tensor("qkw", [SEQ, DQ + DKV], F32, kind="ExternalInput")
    if has_anw:
        anw0 = nc.dram_tensor("anw0", [1, DIM], F32, kind="ExternalInput")
        anwd = nc.dram_tensor("anwd", [1, DIM], F32, kind="ExternalInput")
        # mask column for this core's final output rows (rank-dependent data)
        mfin = nc.dram_tensor("mfin", [128, NG], F32, kind="ExternalInput")

    out_dram = nc.dram_tensor("out", [NG, 128, DIM], F32, kind="ExternalOutput")

    MUL = mybir.AluOpType.mult
    ADD = mybir.AluOpType.add

    with tile.TileContext(nc) as tc:
        with ExitStack() as ctx:
            const = ctx.enter_context(tc.tile_pool(name="const", bufs=1))
            persist = ctx.enter_context(tc.tile_pool(name="persist", bufs=1))
            dram = ctx.enter_context(tc.tile_pool(name="dram", bufs=1, space="DRAM"))

            ident = const.tile([128, 128], F32)
            make_identity(nc, ident[:])
            ones_f = const.tile([128, 1], F32)
            nc.gpsimd.memset(ones_f[:], 1.0)
            ones_col = const.tile([128, 1], ATT)
            nc.scalar.copy(ones_col[:], ones_f[:])
            mpc_sb = const.tile([128, NT], F32)
            nc.sync.dma_start(mpc_sb[:], mpc[:, :])
            eps_q = const.tile([128, 1], F32)
            nc.gpsimd.memset(eps_q[:], float(128.0 * EPS))
            eps_1 = const.tile([128, 1], F32)
            nc.gpsimd.memset(eps_1[:], float(EPS))
            ones_row = const.tile([1, 128], F32)
            nc.gpsimd.memset(ones_row[:], 1.0)
            ATTM = {"bf16": BF16, "f32r": F32}[ATT_DT_NAME]
            mrow_sb = const.tile([1, SEQ], BF16)
            nc.sync.dma_start(mrow_sb[:], mrow[:, :])
            mask_bcast = const.tile([128, SEQ], BF16)
            nc.gpsimd.partition_broadcast(mask_bcast[:], mrow_sb[:])
            dmasks = const.tile([128, 4, 512], ATTM)
            nc.gpsimd.memset(dmasks[:], 1.0)
            for dj in range(4):
                # keep where f - p - 128*dj >= 0 else 0
                nc.gpsimd.affine_select(
                    out=dmasks[:, dj, :], in_=dmasks[:, dj, :],
                    compare_op=mybir.AluOpType.is_ge, fill=0.0,
                    base=-128 * dj, channel_multiplier=-1, pattern=[[1, 512]])

            # persistent activation buffers
            QT = persist.tile([128, HQC, SEQ], ATT)     # q^T per head (hd, tok)
            KTb = persist.tile([128, HKC, SEQ], ATT)    # k^T per kv head
            Vb = persist.tile([128, NT, DKV], ATT)      # v natural (tok, hd)

            # ---------------- Phase 1: QKV projection + norms + rope ---------
            with ExitStack() as p1:
                wpool = p1.enter_context(tc.tile_pool(name="wpool", bufs=1))
                ropep = p1.enter_context(tc.tile_pool(name="ropep", bufs=1))
                xpool = p1.enter_context(tc.tile_pool(name="xpool", bufs=3))
                qkps = p1.enter_context(tc.tile_pool(name="qkps", bufs=1, space="PSUM"))
                tps = p1.enter_context(tc.tile_pool(name="tps", bufs=4, space="PSUM"))
                work = p1.enter_context(tc.tile_pool(name="work", bufs=2))
                if has_qkw:
                    qkwpool = p1.enter_context(tc.tile_pool(name="qkwpool", bufs=2))

                w0_sb = wpool.tile([128, KT, DQ + 2 * DKV], BF16)
                wd_sb = wpool.tile([128, KT, DQ + 2 * DKV], BF16)
                w0_r = w0.ap().rearrange("(k p) f -> p k f", p=128)
                wd_r = wd.ap().rearrange("(k p) f -> p k f", p=128)
                for k in range(KT):
                    nc.gpsimd.dma_start(w0_sb[:, k, :], w0_r[:, k, :])
                    nc.gpsimd.dma_start(wd_sb[:, k, :], wd_r[:, k, :])
                cos_sb = ropep.tile([128, NT, HD], F32)
                nc.gpsimd.dma_start(cos_sb[:], cosf.ap().rearrange("(t p) d -> p t d", p=128))
                sin_sb = ropep.tile([128, NT, HD], F32)
                nc.gpsimd.dma_start(sin_sb[:], sinf.ap().rearrange("(t p) d -> p t d", p=128))

                for T in range(NT):
                    xt = xpool.tile([128, KT, 128], BF16, tag="xt")
                    nc.sync.dma_start(
                        xt[:], xT.ap()[:, ts(T, 128)].rearrange("(k p) n -> p k n", p=128))
                    xm = xpool.tile([128, KT, 128], BF16, tag="xm")
                    nc.vector.tensor_mul(
                        xm[:], xt[:],
                        mask_bcast[:, ts(T, 128)].rearrange("p (o n) -> p o n", o=1).broadcast_to([128, KT, 128]))

                    pq = qkps.tile([128, 512], F32, tag="pq", bufs=2)
                    pkv = qkps.tile([128, 512], F32, tag="pkv", bufs=2)
                    for k in range(KT):
                        st = k == 0
                        sp = k == KT - 1
                        nc.tensor.matmul(pq[:], xt[:, k, :], w0_sb[:, k, 0:512], start=st, stop=False)
                        nc.tensor.matmul(pq[:], xm[:, k, :], wd_sb[:, k, 0:512], start=False, stop=sp)
                        nc.tensor.matmul(pkv[:], xt[:, k, :], w0_sb[:, k, 512:1024], start=st, stop=False)
                        nc.tensor.matmul(pkv[:], xm[:, k, :], wd_sb[:, k, 512:1024], start=False, stop=sp)

                    # rms stats straight from PSUM via ACT Square + accum
                    msq_q = work.tile([128, 4], F32, tag="msq_q")
                    scr = work.tile([128, 128], F32, tag="scr")
                    for h in range(HQC):
                        nc.scalar.activation(
                            scr[:], pq[:, ts(h, 128)],
                            mybir.ActivationFunctionType.Square,
                            accum_out=msq_q[:, h:h + 1])
                    msq_k = work.tile([128, 2], F32, tag="msq_k")
                    for h in range(HKC):
                        nc.scalar.activation(
                            scr[:], pkv[:, ts(h, 128)],
                            mybir.ActivationFunctionType.Square,
                            accum_out=msq_k[:, h:h + 1])
                    sq_q = work.tile([128, 4], F32, tag="sq_q")
                    nc.scalar.activation(sq_q[:], msq_q[:],
                                         mybir.ActivationFunctionType.Sqrt,
                                         scale=1.0, bias=eps_q[:])
                    rs_q = work.tile([128, 4], F32, tag="rs_q")
                    nc.vector.reciprocal(rs_q[:], sq_q[:])
                    sq_k = work.tile([128, 2], F32, tag="sq_k")
                    nc.scalar.activation(sq_k[:], msq_k[:],
                                         mybir.ActivationFunctionType.Sqrt,
                                         scale=1.0 / 128.0, bias=eps_1[:])
                    rs_k = work.tile([128, 2], F32, tag="rs_k")
                    nc.vector.reciprocal(rs_k[:], sq_k[:])

                    # v: plain evict (already modality-selected in PSUM)
                    nc.scalar.copy(Vb[:, T, :], pkv[:, 256:512])

                    if has_qkw:
                        qkw_t = qkwpool.tile([128, DQ + DKV], F32, tag="qkw")
                        nc.sync.dma_start(qkw_t[:], qkw.ap()[ts(T, 128), :])
                        q_src = work.tile([128, 512], F32, tag="q_src")
                        nc.vector.tensor_mul(q_src[:], pq[:], qkw_t[:, 0:DQ])
                        k_src = work.tile([128, 256], F32, tag="k_src")
                        nc.vector.tensor_mul(k_src[:], pkv[:, 0:256], qkw_t[:, DQ:DQ + DKV])
                    else:
                        q_src = pq
                        k_src = pkv

                    # rope (de-interleaved hd: [even dims | odd dims])
                    # rope (de-interleaved hd: [even dims | odd dims])
                    cos_t = cos_sb[:, T, :]
                    sin_t = sin_sb[:, T, :]

                    def rope(dst, src, rs, h):
                        base = src[:, ts(h, 128)]
                        t1 = work.tile([128, 128], F32, tag="rope_t1")
                        nc.vector.scalar_tensor_tensor(
                            out=t1[:], in0=base, scalar=rs[:, h:h + 1], in1=cos_t,
                            op0=MUL, op1=MUL)
                        t2 = work.tile([128, 128], F32, tag="rope_t2")
                        nc.vector.scalar_tensor_tensor(
                            out=t2[:, 0:64], in0=base[:, 64:128], scalar=rs[:, h:h + 1],
                            in1=sin_t[:, 0:64], op0=MUL, op1=MUL)
                        nc.vector.scalar_tensor_tensor(
                            out=t2[:, 64:128], in0=base[:, 0:64], scalar=rs[:, h:h + 1],
                            in1=sin_t[:, 64:128], op0=MUL, op1=MUL)
                        nc.vector.tensor_add(dst[:, ts(h, 128)], t1[:], t2[:])

                    q_rot = work.tile([128, 512], F32, tag="q_rot")
                    for h in range(HQC):
                        rope(q_rot, q_src, rs_q, h)
                    k_rot = work.tile([128, 256], F32, tag="k_rot")
                    for h in range(HKC):
                        rope(k_rot, k_src, rs_k, h)

                    # transpose to (hd, tok) layouts
                    for h in range(HQC):
                        tp = tps.tile([128, 128], F32, tag="tp")
                        nc.tensor.transpose(tp[:], q_rot[:, ts(h, 128)], ident[:])
                        nc.scalar.copy(QT[:, h, ts(T, 128)], tp[:])
                    for h in range(HKC):
                        tp = tps.tile([128, 128], F32, tag="tp")
                        nc.tensor.transpose(tp[:], k_rot[:, ts(h, 128)], ident[:])
                        nc.scalar.copy(KTb[:, h, ts(T, 128)], tp[:])

            if STAGE == "p1":
                with ExitStack() as pdbg:
                    dbg = pdbg.enter_context(tc.tile_pool(name="dbg", bufs=2))
                    for g in range(NG):
                        t_f = dbg.tile([128, DIM], F32, tag="t_f")
                        nc.scalar.copy(t_f[:], QT[:, g, :])
                        nc.sync.dma_start(out_dram[g], t_f[:])

            # ---------------- Phase 2+3: attention + wo + RS + final norm ----
            with ExitStack() as p23:
                wopool = p23.enter_context(tc.tile_pool(name="wopool", bufs=1))
                ofp = p23.enter_context(tc.tile_pool(name="ofp", bufs=1))
                sps = p23.enter_context(tc.tile_pool(name="sps", bufs=2, space="PSUM"))
                otps = p23.enter_context(tc.tile_pool(name="otps", bufs=2, space="PSUM"))
                dnps = p23.enter_context(tc.tile_pool(name="dnps", bufs=1, space="PSUM"))
                dbps = p23.enter_context(tc.tile_pool(name="dbps", bufs=1, space="PSUM"))
                wops = p23.enter_context(tc.tile_pool(name="wops", bufs=2, space="PSUM"))
                probs = p23.enter_context(tc.tile_pool(name="probs", bufs=6))
                att = p23.enter_context(tc.tile_pool(name="att", bufs=2))
                opool = p23.enter_context(tc.tile_pool(name="opool", bufs=2))
                npool = p23.enter_context(tc.tile_pool(name="npool", bufs=2))

                ofT = ofp.tile([128, HQC, SEQ], BF16)   # out_flat^T (hd, tok)
                ofm = ofp.tile([128, HQC, SEQ], BF16)   # masked out_flat^T

                wo0_sb = wopool.tile([128, 4, DIM], BF16)
                nc.sync.dma_start(wo0_sb[:], wo0.ap().rearrange("(k p) f -> p k f", p=128))
                wod_sb = wopool.tile([128, 4, DIM], BF16)
                nc.sync.dma_start(wod_sb[:], wod.ap().rearrange("(k p) f -> p k f", p=128))
                if has_anw:
                    anw0_sb = wopool.tile([1, DIM], F32)
                    nc.sync.dma_start(anw0_sb[:], anw0[:, :])
                    anwd_sb = wopool.tile([1, DIM], F32)
                    nc.sync.dma_start(anwd_sb[:], anwd[:, :])
                    anw0_b = wopool.tile([128, DIM], F32)
                    nc.gpsimd.partition_broadcast(anw0_b[:], anw0_sb[:])
                    anwd_b = wopool.tile([128, DIM], F32)
                    nc.gpsimd.partition_broadcast(anwd_b[:], anwd_sb[:])
                    mfin_sb = wopool.tile([128, NG], F32)
                    nc.sync.dma_start(mfin_sb[:], mfin[:, :])

                if STAGE == "p1":
                    loop_gs = []
                else:
                    loop_gs = list(range(NG))
                pending_rs = []
                pending_den = []

                def do_den(g, h, dn_ps):
                    den = att.tile([1, 512], F32, tag="den")
                    nc.vector.reciprocal(den[:], dn_ps[:])
                    db_ps = dbps.tile([128, 512], F32, tag="db")
                    nc.tensor.matmul(db_ps[:], ones_row[:], den[:],
                                     start=True, stop=True)
                    den_b = att.tile([128, 512], F32, tag="den_b")
                    nc.scalar.copy(den_b[:], db_ps[:])
                    nc.vector.tensor_mul(
                        ofT[:, h, ts(g, 512)], ofT[:, h, ts(g, 512)], den_b[:])
                    nc.vector.tensor_mul(
                        ofm[:, h, ts(g, 512)], ofT[:, h, ts(g, 512)],
                        mask_bcast[:, ts(g, 512)])

                def do_final_norm(g, rs_out, dep=None):
                    sum_sb = npool.tile([128, DIM], F32, tag="sum_sb")
                    first = nc.sync.dma_start(sum_sb[:], rs_out[:])
                    if dep is not None:
                        _add_dep_helper(first.ins, dep.ins, sync=False,
                                        reason="defer norm past next chunk")
                    fin = npool.tile([128, DIM], F32, tag="fin")
                    z = npool.tile([128, 1], F32, tag="z")
                    nc.vector.scalar_tensor_tensor(
                        out=fin[:], in0=sum_sb[:], scalar=1.0, in1=sum_sb[:],
                        op0=MUL, op1=MUL, accum_out=z[:])
                    sz = npool.tile([128, 1], F32, tag="sz")
                    nc.scalar.activation(sz[:], z[:],
                                         mybir.ActivationFunctionType.Sqrt,
                                         scale=1.0 / float(DIM), bias=eps_1[:])
                    rz = npool.tile([128, 1], F32, tag="rz")
                    nc.vector.reciprocal(rz[:], sz[:])
                    nc.scalar.mul(fin[:], sum_sb[:], rz[:])
                    if has_anw:
                        # anw_sel[tok, d] = anw0[d] + m[tok] * anwd[d]
                        anw_sel = npool.tile([128, DIM], F32, tag="anw_sel")
                        nc.vector.scalar_tensor_tensor(
                            out=anw_sel[:], in0=anwd_b[:], scalar=mfin_sb[:, g:g + 1],
                            in1=anw0_b[:], op0=MUL, op1=ADD)
                        nc.vector.tensor_mul(fin[:], fin[:], anw_sel[:])
                    nc.sync.dma_start(out_dram[g], fin[:])

                for g in loop_gs:
                    for h in range(HQC):
                        kv = h // (HQC // HKC)
                        njt = 4 * (g + 1)
                        ot_ps = otps.tile([128, 512], F32, tag="ot")
                        dn_ps = dnps.tile([1, 512], F32, tag="dn")
                        for j in range(njt):
                            s_ps = sps.tile([128, 512], F32, tag="s")
                            nc.tensor.matmul(
                                s_ps[:], KTb[:, kv, ts(j, 128)],
                                QT[:, h, ts(g, 512)], start=True, stop=True)
                            p_t = probs.tile([128, 512], ATT, tag="p")
                            nc.scalar.activation(
                                p_t[:], s_ps[:], mybir.ActivationFunctionType.Exp)
                            if j >= 4 * g:
                                pm_t = probs.tile([128, 512], ATT, tag="pm")
                                nc.vector.tensor_mul(
                                    pm_t[:], p_t[:], dmasks[:, j - 4 * g, :])
                                p_t = pm_t
                            st = j == 0
                            sp = j == njt - 1
                            nc.tensor.matmul(
                                ot_ps[:], Vb[:, j, ts(kv, 128)], p_t[:],
                                start=st, stop=sp)
                            nc.tensor.matmul(
                                dn_ps[:], ones_col[:], p_t[:],
                                start=st, stop=sp)
                        # fast raw evict frees the psum; normalization is
                        # deferred one head so PE never waits on the den chain
                        nc.scalar.copy(ofT[:, h, ts(g, 512)], ot_ps[:])
                        pending_den.append((h, dn_ps))
                        if len(pending_den) > 1:
                            do_den(g, *pending_den.pop(0))

                    if STAGE == "p2":
                        t_f2 = npool.tile([128, DIM], F32, tag="t_f2")
                        nc.scalar.copy(t_f2[:], ofT[:, 0, :])
                        nc.sync.dma_start(out_dram[g], t_f2[:])
                        continue

                    while pending_den:
                        do_den(g, *pending_den.pop(0))

                    # wo projection for this 512-token chunk; the modality mask
                    # commutes through the matmul (per-token = lhsT free dim),
                    # so select happens at eviction with a per-partition scalar.
                    rs_in = dram.tile([512, DIM], F32, tag="rs_in", bufs=2)
                    for u in range(4):
                        T = 4 * g + u
                        o_sb = opool.tile([128, DIM], F32, tag="o_sb")
                        for n in range(4):
                            wo_ps = wops.tile([128, 512], F32, tag="wop")
                            for kk in range(4):
                                nc.tensor.matmul(
                                    wo_ps[:], ofT[:, kk, ts(T, 128)],
                                    wo0_sb[:, kk, ts(n, 512)],
                                    start=(kk == 0), stop=False)
                                nc.tensor.matmul(
                                    wo_ps[:], ofm[:, kk, ts(T, 128)],
                                    wod_sb[:, kk, ts(n, 512)],
                                    start=False, stop=(kk == 3))
                            nc.scalar.copy(o_sb[:, ts(n, 512)], wo_ps[:])
                        last_rsin_dma = nc.sync.dma_start(rs_in[ts(u, 128), :], o_sb[:])

                    rs_out = dram.tile([128, DIM], F32, tag="rs_out", bufs=2)
                    if STAGE == "p3nc":
                        nc.sync.dma_start(rs_out[:], rs_in[0:128, :])
                    else:
                        nc.gpsimd.collective_compute(
                            "ReduceScatter", mybir.AluOpType.add,
                            replica_groups=GROUPS,
                            ins=[rs_in.opt()], outs=[rs_out.opt()])
                    pending_rs.append((g, rs_out))
                    # final norm for the PREVIOUS chunk: its RS finished while
                    # this chunk computed, so the queues never block on it
                    if len(pending_rs) > 1:
                        pg, prs = pending_rs.pop(0)
                        do_final_norm(pg, prs, dep=last_rsin_dma)

                for item in pending_rs:
                    do_final_norm(*item)

    nc.compile()
    return nc


def _prep_inputs(x, freqs_cos, freqs_sin, wq, wk, wv, wo,
                 q_norm_w, k_norm_w, attn_norm_w, modality_ids,
                 has_qkw, has_anw):
    """Build the 8 per-core input maps (numpy marshaling only)."""
    x = np.asarray(x, np.float32)
    freqs_cos = np.asarray(freqs_cos, np.float32)
    freqs_sin = np.asarray(freqs_sin, np.float32)
    wq = np.asarray(wq, np.float32)
    wk = np.asarray(wk, np.float32)
    wv = np.asarray(wv, np.float32)
    wo = np.asarray(wo, np.float32)
    mids = np.asarray(modality_ids).reshape(BS, SEQ)

    # de-interleave the hd dimension: [even dims, odd dims]
    perm = np.concatenate([np.arange(0, HD, 2), np.arange(1, HD, 2)])

    def permute_heads(w, nh):
        w4 = w.reshape(E, DIM, nh, HD)
        return w4[:, :, :, perm].reshape(E, DIM, nh * HD)

    wq_p = permute_heads(wq, HQ)
    wk_p = permute_heads(wk, HK)
    wv_p = permute_heads(wv, HK)   # v permuted too; wo rows permuted to match
    wo4 = wo.reshape(E, HQ, HD, DIM)[:, :, perm, :].reshape(E, HQ * HD, DIM)

    cosf = np.concatenate([freqs_cos, freqs_cos], axis=1)          # (SEQ, HD)
    sinf = np.concatenate([-freqs_sin, freqs_sin], axis=1)         # (SEQ, HD)

    in_maps = []
    for c in range(N_CORES):
        b, r = divmod(c, TP)
        qs = slice(r * DQ, (r + 1) * DQ)
        ks = slice(r * DKV, (r + 1) * DKV)
        w0c = np.concatenate([wq_p[0][:, qs], wk_p[0][:, ks], wv_p[0][:, ks]], axis=1)
        w1c = np.concatenate([wq_p[1][:, qs], wk_p[1][:, ks], wv_p[1][:, ks]], axis=1)
        wo0c = wo4[0][r * DQ:(r + 1) * DQ, :]
        wo1c = wo4[1][r * DQ:(r + 1) * DQ, :]
        m = mids[b].astype(np.float32)
        im = {
            "xT": np.ascontiguousarray(x[b].T).astype(ml_dtypes.bfloat16),
            "w0": w0c.astype(ml_dtypes.bfloat16),
            "wd": (w1c - w0c).astype(ml_dtypes.bfloat16),
            "wo0": wo0c.astype(ml_dtypes.bfloat16),
            "wod": (wo1c - wo0c).astype(ml_dtypes.bfloat16),
            "cosf": cosf,
            "sinf": sinf,
            "mpc": np.ascontiguousarray(m.reshape(NT, 128).T),
            "mrow": m.reshape(1, SEQ).astype(ml_dtypes.bfloat16),
        }
        if has_qkw:
            qw = np.asarray(q_norm_w, np.float32)[:, perm]   # (E, HD)
            kw = np.asarray(k_norm_w, np.float32)[:, perm]
            qsel = qw[mids[b]]                               # (SEQ, HD)
            ksel = kw[mids[b]]
            im["qkw"] = np.concatenate(
                [np.tile(qsel, (1, HQC)), np.tile(ksel, (1, HKC))], axis=1)
        if has_anw:
            aw = np.asarray(attn_norm_w, np.float32)
            im["anw0"] = np.ascontiguousarray(aw[0:1])
            im["anwd"] = (aw[1] - aw[0]).reshape(1, DIM).copy()
            mf = np.empty((128, NG), np.float32)
            for g in range(NG):
                t0 = 512 * g + 128 * r
                mf[:, g] = m[t0:t0 + 128]
            im["mfin"] = mf
        in_maps.append(im)
    return in_maps


def kernel(**inputs):
    q_norm_w = np.asarray(inputs["q_norm_w"], np.float32)
    k_norm_w = np.asarray(inputs["k_norm_w"], np.float32)
    attn_norm_w = np.asarray(inputs["attn_norm_w"], np.float32)
    has_qkw = not (np.all(q_norm_w == 1.0) and np.all(k_norm_w == 1.0))
    has_anw = not np.all(attn_norm_w == 1.0)

    key = (has_qkw, has_anw)
    if key not in _BUILD_CACHE:
        _BUILD_CACHE[key] = build_nc(has_qkw, has_anw)
    nc = _BUILD_CACHE[key]

    in_maps = _prep_inputs(
        inputs["x"], inputs["freqs_cos"], inputs["freqs_sin"],
        inputs["wq"], inputs["wk"], inputs["wv"], inputs["wo"],
        q_norm_w, k_norm_w, attn_norm_w, inputs["modality_ids"],
        has_qkw, has_anw)

    res = run_bass_kernel_spmd(nc, in_maps, core_ids=list(range(N_CORES)))

    out = np.empty((BS, SEQ, DIM), np.float32)
    for c in range(N_CORES):
        b, r = divmod(c, TP)
        oc = res.results[c]["out"]          # (NG, 128, DIM)
        for g in range(NG):
            t0 = 512 * g + 128 * r
            out[b, t0:t0 + 128, :] = oc[g]
    return out
